# revision 25
# baseline (speedup 1.0000x reference)
"""AxialSelfAttention2d Trainium2 kernel (8 NeuronCores), bf16 compute.

Sharding: stage 1 (row attention, attends along L) is S-sharded (32 rows/core);
stage 2 (column attention, attends along S) is L-sharded (32 cols/core).
Between stages a single bf16 AllToAll reshards out1 = x + row_out.

Per-core stage structure (identical for both stages; "rows" = s for stage 1,
l for stage 2; free axis = the 256-long attended axis):
  - The stage input x lives fully in SBUF ([128, 8192] bf16 per 128-channel
    chunk); stage 2 reuses stage 1's x slots (Tile tag reuse).
  - QKV 1x1-conv projection as bf16 matmuls; q/k biases folded into the
    mandatory PSUM->SBUF copies (ACT Identity+bias for q, DVE tensor_scalar
    add for k); the v bias rides the residual add (softmax weights sum to 1,
    so adding bv after normalization is exact).
  - Per (row, head-pair): QK^T for both heads into one 2-bank PSUM tile
    [128, 1024] (row-tiled on the PE: head A uses array rows 0-63, head B
    64-127), one batched exp on ACT, then AV + softmax denominators for both
    heads into one [128, 512] PSUM tile (denominators broadcast 64-wide by an
    all-ones lhsT so the DVE reciprocal+normalize run at full width covering
    both heads in two ops).
  - Residual+bias folded in by Pool scalar_tensor_tensor adds.
Output y is [512, 32, 256] bf16 per core, (c, l, s) order; the host
transposes/concats/casts to the final fp32 [1, 512, 256, 256].
"""

import numpy as np
import concourse.bass as bass
import concourse.tile as tile
import concourse.mybir as mybir
from concourse import bacc

N_CORES = 8
D = 512                 # embed channels
H = 8                   # heads
DH = 64                 # head dim
RLOC = 32               # rows per core (s-rows stage 1, l-cols stage 2)
PIX = RLOC * 256        # 8192 pixels per core per stage
F32 = mybir.dt.float32
BF16 = mybir.dt.bfloat16
ADD = mybir.AluOpType.add
IDENT = mybir.ActivationFunctionType.Identity
EXP = mybir.ActivationFunctionType.Exp

_CACHE = {}


def _load_weights(nc, sb, prefix, w_ins):
    """DMA weight/bias DRAM inputs into SBUF tiles. Returns dict of tiles."""
    wq_d, wk_d, wv_d, bq_d, bk_d, bv_d = w_ins
    out = {}
    for wname, wd in (("wq", wq_d), ("wk", wk_d), ("wv", wv_d)):
        tiles = []
        for c4 in range(4):
            t = sb.tile([128, 512], BF16, name=f"{prefix}{wname}{c4}", bufs=1)
            nc.sync.dma_start(t[:], wd[c4 * 128:(c4 + 1) * 128, :])
            tiles.append(t)
        out[wname] = tiles
    for bname, bd in (("bq", bq_d), ("bk", bk_d)):
        tiles = []
        for oc in range(4):
            t = sb.tile([128, 1], F32, name=f"{prefix}{bname}{oc}", bufs=1)
            nc.sync.dma_start(t[:], bd[oc * 128:(oc + 1) * 128, :])
            tiles.append(t)
        out[bname] = tiles
    bvr = sb.tile([1, 512], BF16, name=f"{prefix}bvr", bufs=1)
    nc.sync.dma_start(bvr[:], bv_d[:])
    out["bvr"] = bvr
    return out


def _stage(nc, sb, ps, w, ones64, xb, stag_for, residual_dst, res_pattern,
           after_chunk, prefix):
    """One attention stage over this core's 32 rows (16 chunks of 2 rows).

    xb: 4 SBUF tiles [128, 8192] bf16 holding this stage's input x.
    stag_for(chunk, r, m) -> dst AP [128, 256] for the normalized output of
      head-pair m (channels m*128:(m+1)*128), row r of the chunk.
    residual_dst(chunk, cc) -> dst AP matching res_pattern's view of x.
    res_pattern: einops pattern for the 2-row x chunk residual view.
    after_chunk(chunk): hook (stores / A2A kick).
    """
    for chunk in range(16):
        xs = [xb[cc][:, chunk * 512:(chunk + 1) * 512] for cc in range(4)]
        # --- q/k projections: out [o-chunk 128, 512 pix], bias on copy ---
        q_sb, k_sb = [], []
        for wname, bname, dst in (("wq", "bq", q_sb), ("wk", "bk", k_sb)):
            for oc in range(4):
                pp = ps.tile([128, 512], F32, name="pp", tag="pp", bufs=2)
                for c4 in range(4):
                    nc.tensor.matmul(
                        pp[:],
                        w[wname][c4][:, oc * 128:(oc + 1) * 128],
                        xs[c4],
                        start=(c4 == 0), stop=(c4 == 3),
                    )
                t = sb.tile([128, 512], BF16, name=f"{wname}o{oc}",
                            tag=f"{wname}o", bufs=5)
                with nc.allow_low_precision(reason="bf16 q/k"):
                    if wname == "wq":
                        nc.scalar.activation(t[:], pp[:], IDENT,
                                             bias=w[bname][oc][:])
                    else:
                        nc.vector.tensor_scalar_add(t[:], pp[:],
                                                    w[bname][oc][:])
                dst.append(t)
        # --- v projected transposed [pix-chunk 128, 8 heads x 64] ---
        vT_sb = []
        for pc in range(4):
            pv = ps.tile([128, 512], F32, name="pp", tag="pp", bufs=2)
            for c4 in range(4):
                nc.tensor.matmul(
                    pv[:],
                    xs[c4][:, pc * 128:(pc + 1) * 128],
                    w["wv"][c4][:],
                    start=(c4 == 0), stop=(c4 == 3),
                )
            t = sb.tile([128, 512], BF16, name=f"vT{pc}", tag="vT", bufs=5)
            with nc.allow_low_precision(reason="bf16 v"):
                nc.vector.tensor_add(t[:], pv[:], w["bvt"][:])
            vT_sb.append(t)
        # --- attention per (row-in-chunk, head-pair) ---
        for r in range(2):
            for m in range(4):
                # QK^T both heads: at2[j', (h2, jh, i)]; K=64 row-tiled
                at2 = ps.tile([128, 1024], F32, name="at2", tag="at2", bufs=2)
                for h2 in range(2):
                    ph = h2 * 64
                    for jh in range(2):
                        nc.tensor.matmul(
                            at2[:, (h2 * 2 + jh) * 256:(h2 * 2 + jh + 1) * 256],
                            k_sb[m][ph:ph + 64,
                                    r * 256 + jh * 128:r * 256 + (jh + 1) * 128],
                            q_sb[m][ph:ph + 64, r * 256:(r + 1) * 256],
                            start=True, stop=True,
                        )
                e2 = sb.tile([128, 1024], BF16, name="e2", tag="e2", bufs=4)
                with nc.allow_low_precision(reason="bf16 attention weights"):
                    nc.scalar.activation(e2[:], at2[:], EXP)
                # AV + denominators for both heads in one [128, 512] bank:
                # rows h2*64.. = head h2; cols 0:256 AV, 256:512 denom
                ob = ps.tile([128, 512], F32, name="ob", tag="ob", bufs=2)
                for h2 in range(2):
                    po = h2 * 64
                    for jh in range(2):
                        nc.tensor.matmul(
                            ob[po:po + 64, 0:256],
                            vT_sb[2 * r + jh][:, (2 * m + h2) * 64:
                                              (2 * m + h2) * 64 + 64],
                            e2[:, (h2 * 2 + jh) * 256:(h2 * 2 + jh + 1) * 256],
                            start=(jh == 0), stop=(jh == 1),
                        )
                    for jh in range(2):
                        nc.tensor.matmul(
                            ob[po:po + 64, 256:512],
                            ones64[:, :],
                            e2[:, (h2 * 2 + jh) * 256:(h2 * 2 + jh + 1) * 256],
                            start=(jh == 0), stop=(jh == 1),
                        )
                r_sb = sb.tile([128, 256], F32, name="r_sb", tag="r_sb",
                               bufs=3)
                nc.vector.reciprocal(r_sb[:], ob[:, 256:512])
                with nc.allow_low_precision(reason="bf16 staging"):
                    nc.vector.tensor_mul(
                        stag_for(chunk, r, m), ob[:, 0:256], r_sb[:])
        # bulk residual on Pool
        for cc in range(4):
            dst = residual_dst(chunk, cc)
            src = xs[cc].rearrange(res_pattern, r=2)
            with nc.allow_low_precision(reason="bf16 staging"):
                nc.gpsimd.tensor_add(dst, dst, src)
        after_chunk(chunk)


def _build(variant="full"):
    ndev = 1 if variant == "sim1" else N_CORES
    nc = bacc.Bacc("TRN2", target_bir_lowering=False, debug=False,
                   num_devices=ndev)
    xi = nc.dram_tensor("xi", [D, PIX], BF16, kind="ExternalInput").ap()
    y = nc.dram_tensor("y", [D, RLOC, 256], BF16, kind="ExternalOutput").ap()
    w_ins = {}
    for p in ("1", "2"):
        ins = []
        for nm, shp, dt in (("wq", [D, D], BF16), ("wk", [D, D], BF16),
                            ("wv", [D, D], BF16), ("bq", [D, 1], F32),
                            ("bk", [D, 1], F32), ("bv", [1, D], BF16)):
            ins.append(nc.dram_tensor(nm + p, shp, dt, kind="ExternalInput").ap())
        w_ins[p] = ins

    with tile.TileContext(nc) as tc:
        with tc.tile_pool(name="sb", bufs=1) as sb, \
             tc.tile_pool(name="psum", bufs=1, space="PSUM") as psp, \
             tc.tile_pool(name="dram", bufs=1, space="DRAM") as dram:
            # A2A blocks: [dest j, c, l32 (j's share), s32 (sender's share)]
            a2a_in = dram.tile([N_CORES, D, RLOC, RLOC], BF16, name="a2a_in")
            a2a_out = dram.tile([N_CORES, D, RLOC, RLOC], BF16, name="a2a_out")

            ones64 = sb.tile([128, 64], BF16, name="ones64", bufs=1)
            nc.gpsimd.memset(ones64[:], 1.0)
            ones_row = sb.tile([1, 128], BF16, name="ones_row", bufs=1)
            nc.gpsimd.memset(ones_row[:], 1.0)
            w1 = _load_weights(nc, sb, "s1", w_ins["1"])
            w2 = _load_weights(nc, sb, "s2", w_ins["2"])
            # broadcast each stage's v-bias down 128 partitions (K=1 matmul)
            for sidx, wd in ((1, w1), (2, w2)):
                bvp = psp.tile([128, 512], F32, name="pp", tag="pp", bufs=2)
                nc.tensor.matmul(bvp[:], ones_row[0:1, :], wd["bvr"][:],
                                 start=True, stop=True)
                bvt = sb.tile([128, 512], BF16, name=f"bvt{sidx}",
                              tag=f"bvt{sidx}", bufs=1)
                with nc.allow_low_precision(reason="bf16 bias"):
                    nc.vector.tensor_copy(bvt[:], bvp[:])
                wd["bvt"] = bvt

            # ---- stage 1: row attention, S-sharded ----
            xb1 = []
            for cc in range(4):
                t = sb.tile([128, PIX], BF16, name=f"xb1_{cc}",
                            tag=f"xb{cc}", bufs=1)
                xb1.append(t)
            for span in range(4):  # span-split so chunk 0 starts early
                for cc in range(4):
                    sl = slice(span * 2048, (span + 1) * 2048)
                    nc.sync.dma_start(xb1[cc][:, sl],
                                      xi[cc * 128:(cc + 1) * 128, sl])
            # staging: per cc a [128, 256 l, 32 s] bf16 tile for the full
            # 32-row block; stored to a2a_in at stage end.
            stag1 = []
            for cc in range(4):
                t = sb.tile([128, 256, RLOC], BF16, name=f"stag1_{cc}",
                            tag=f"stg{cc}", bufs=1)
                stag1.append(t)

            def stag_for1(chunk, r, m):
                return stag1[m][:, :, chunk * 2 + r]

            def residual_dst1(chunk, cc):
                return stag1[cc][:, :, chunk * 2:chunk * 2 + 2]

            def after_chunk1(chunk):
                if chunk != 15:
                    return
                for cc in range(4):
                    for j in range(N_CORES):
                        nc.scalar.dma_start(
                            a2a_in[j, cc * 128:(cc + 1) * 128, :, :],
                            stag1[cc][:, j * 32:(j + 1) * 32, :])
                if variant == "sim1":
                    nc.gpsimd.dma_start(a2a_out[:], a2a_in[:])
                else:
                    nc.gpsimd.collective_compute(
                        "AllToAll", mybir.AluOpType.bypass,
                        replica_groups=[list(range(N_CORES))],
                        ins=[a2a_in.opt()], outs=[a2a_out.opt()],
                    )

            _stage(nc, sb, psp, w1, ones64, xb1, stag_for1, residual_dst1,
                   "c (r l) -> c l r", after_chunk1, "s1")

            # ---- stage 2: column attention, L-sharded ----
            # x2 reuses the xb slots; pix layout [c, (l32, s256)]
            xb2 = []
            for cc in range(4):
                t = sb.tile([128, PIX], BF16, name=f"xb2_{cc}",
                            tag=f"xb{cc}", bufs=1)
                xb2.append(t)
            for span in range(4):      # l-spans of 8, early spans land first
                for cc in range(4):
                    for i in range(N_CORES):
                        nc.sync.dma_start(
                            xb2[cc][:].rearrange("c (l s) -> c l s", l=RLOC)
                            [:, span * 8:(span + 1) * 8, i * 32:(i + 1) * 32],
                            a2a_out[i, cc * 128:(cc + 1) * 128,
                                    span * 8:(span + 1) * 8, :])

            # staging: per (cc, g) a [128, 8 l, 256 s] tile (reuses the big
            # stg slots), stored to y after every 4 chunks.
            stag2 = [[None] * 4 for _ in range(4)]

            def stag_for2(chunk, r, m):
                g = chunk // 4
                if stag2[m][g] is None:
                    for cc in range(4):
                        stag2[cc][g] = sb.tile(
                            [128, 8, 256], BF16, name=f"stag2_{cc}",
                            tag=f"stg{cc}", bufs=1)
                return stag2[m][g][:, (chunk % 4) * 2 + r, :]

            def residual_dst2(chunk, cc):
                g = chunk // 4
                return stag2[cc][g][:, (chunk % 4) * 2:(chunk % 4) * 2 + 2, :]

            def after_chunk2(chunk):
                if chunk % 4 != 3:
                    return
                g = chunk // 4
                for cc in range(4):
                    nc.scalar.dma_start(
                        y[cc * 128:(cc + 1) * 128, g * 8:(g + 1) * 8, :],
                        stag2[cc][g][:])

            _stage(nc, sb, psp, w2, ones64, xb2, stag_for2, residual_dst2,
                   "c (r s) -> c r s", after_chunk2, "s2")

    nc.compile()
    return nc


def _get_nc(variant="full"):
    key = "nc:" + variant
    if key not in _CACHE:
        _CACHE[key] = _build(variant)
    return _CACHE[key]


def _in_maps(x, Wr, br, Wc, bc):
    import ml_dtypes
    bf = ml_dtypes.bfloat16
    x = np.asarray(x, dtype=np.float32)
    stage_w = {}
    for p, W, b in (("1", np.asarray(Wr, np.float32), np.asarray(br, np.float32)),
                    ("2", np.asarray(Wc, np.float32), np.asarray(bc, np.float32))):
        stage_w["wq" + p] = np.ascontiguousarray(W[0:D].T).astype(bf)
        stage_w["wk" + p] = np.ascontiguousarray(W[D:2 * D].T).astype(bf)
        stage_w["wv" + p] = np.ascontiguousarray(W[2 * D:3 * D].T).astype(bf)
        stage_w["bq" + p] = np.ascontiguousarray(b[0:D].reshape(D, 1))
        stage_w["bk" + p] = np.ascontiguousarray(b[D:2 * D].reshape(D, 1))
        stage_w["bv" + p] = np.ascontiguousarray(
            b[2 * D:3 * D].reshape(1, D)).astype(bf)
    maps = []
    for i in range(N_CORES):
        m = {"xi": np.ascontiguousarray(
            x[0, :, i * RLOC:(i + 1) * RLOC, :].reshape(D, PIX)).astype(bf)}
        m.update(stage_w)
        maps.append(m)
    return maps


def _get_runner(variant="full"):
    """Build (once) a cached jitted shard_map callable over the 8 cores."""
    rkey = "runner:" + variant
    if rkey in _CACHE:
        return _CACHE[rkey]
    import jax
    from jax.sharding import Mesh, PartitionSpec
    from jax.experimental.shard_map import shard_map
    from concourse import bass2jax as b2j

    nc = _get_nc(variant)
    b2j.install_neuronx_cc_hook()
    part_name = nc.partition_id_tensor.name if nc.partition_id_tensor else None
    in_names, out_names, out_avals, zero_outs = [], [], [], []
    for alloc in nc.m.functions[0].allocations:
        if not isinstance(alloc, mybir.MemoryLocationSet):
            continue
        name = alloc.memorylocations[0].name
        if alloc.kind == "ExternalInput":
            if name != part_name:
                in_names.append(name)
        elif alloc.kind == "ExternalOutput":
            out_names.append(name)
            shape = tuple(alloc.tensor_shape)
            dtype = mybir.dt.np(alloc.dtype)
            out_avals.append(jax.core.ShapedArray(shape, dtype))
            zero_outs.append(np.zeros(shape, dtype))
    n_params = len(in_names)
    all_names = in_names + out_names
    if part_name is not None:
        all_names = all_names + [part_name]

    def _body(*args):
        operands = list(args)
        if part_name is not None:
            operands.append(b2j.partition_id_tensor())
        outs = b2j._bass_exec_p.bind(
            *operands,
            out_avals=tuple(out_avals),
            in_names=tuple(all_names),
            out_names=tuple(out_names),
            lowering_input_output_aliases=(),
            sim_require_finite=True,
            sim_require_nnan=True,
            nc=nc,
        )
        return tuple(outs)

    devices = jax.devices()[:N_CORES]
    mesh = Mesh(np.asarray(devices), ("core",))
    specs = (PartitionSpec("core"),) * (n_params + len(out_names))
    sharded = jax.jit(
        shard_map(_body, mesh=mesh, in_specs=specs,
                  out_specs=(PartitionSpec("core"),) * len(out_names),
                  check_rep=False),
        keep_unused=True,
    )
    concat_zeros = [
        jax.device_put(
            np.zeros((N_CORES * z.shape[0], *z.shape[1:]), z.dtype),
            jax.sharding.NamedSharding(mesh, PartitionSpec("core")))
        for z in zero_outs
    ]
    _CACHE[rkey] = (sharded, in_names, out_names, out_avals, concat_zeros)
    return _CACHE[rkey]


def _run(maps):
    sharded, in_names, out_names, out_avals, concat_zeros = _get_runner()
    concat_in = [
        np.concatenate([maps[c][nm] for c in range(N_CORES)], axis=0)
        for nm in in_names
    ]
    out_arrs = sharded(*concat_in, *concat_zeros)
    return [
        {nm: np.asarray(out_arrs[i]).reshape(N_CORES, *out_avals[i].shape)[c]
         for i, nm in enumerate(out_names)}
        for c in range(N_CORES)
    ]


def kernel(x, Wr, br, Wc, bc):
    maps = _in_maps(x, Wr, br, Wc, bc)
    results = _run(maps)
    # per-core y is [c, l32, s256] bf16; transpose to [c, s, l] and concat l
    cols = [np.transpose(results[i]["y"].astype(np.float32), (0, 2, 1))
            for i in range(N_CORES)]
    out = np.concatenate(cols, axis=2)
    return out[None]


# revision 32
# speedup vs baseline: 1.5342x; 1.5342x over previous
"""AxialSelfAttention2d Trainium2 kernel (8 NeuronCores), bf16 compute.

Sharding: stage 1 (row attention, attends along L) is S-sharded (32 rows/core);
stage 2 (column attention, attends along S) is L-sharded (32 cols/core).
Between stages a single bf16 AllToAll reshards out1 = x + row_out.

Per-core stage structure (identical for both stages; "rows" = s for stage 1,
l for stage 2; free axis = the 256-long attended axis):
  - QKV 1x1-conv projection as bf16 matmuls; q/k biases folded into the
    mandatory PSUM->SBUF copies (ACT Identity+bias for q, DVE tensor_scalar
    add for k); the v bias folded into the v PSUM->SBUF copy as a DVE
    tensor_add with a per-stage [128, 512] broadcast tile (built once by a
    K=1 ones x bv matmul).
  - Per (row, head-pair): QK^T for both heads into one 2-bank PSUM tile
    [128, 1024] (row-tiled on the PE: head A uses array rows 0-63, head B
    64-127), one batched exp on ACT, then AV + softmax denominators for both
    heads into one [128, 512] PSUM tile (denominators broadcast 64-wide by an
    all-ones lhsT so the DVE reciprocal+normalize run at full width covering
    both heads in two ops).
  - Residual+bias folded in by Pool scalar_tensor_tensor adds.
Output y is [512, 32, 256] bf16 per core, (c, l, s) order; the host
transposes/concats/casts to the final fp32 [1, 512, 256, 256].
"""

import numpy as np
import concourse.bass as bass
import concourse.tile as tile
import concourse.mybir as mybir
from concourse import bacc

N_CORES = 8
D = 512                 # embed channels
H = 8                   # heads
DH = 64                 # head dim
RLOC = 32               # rows per core (s-rows stage 1, l-cols stage 2)
PIX = RLOC * 256        # 8192 pixels per core per stage
F32 = mybir.dt.float32
BF16 = mybir.dt.bfloat16
IDENT = mybir.ActivationFunctionType.Identity
EXP = mybir.ActivationFunctionType.Exp

_CACHE = {}


def _load_weights(nc, sb, prefix, w_ins):
    """DMA weight/bias DRAM inputs into SBUF tiles. Returns dict of tiles."""
    wq_d, wk_d, wv_d, bq_d, bk_d, bv_d = w_ins
    out = {}
    for wname, wd in (("wq", wq_d), ("wk", wk_d), ("wv", wv_d)):
        tiles = []
        for c4 in range(4):
            t = sb.tile([128, 512], BF16, name=f"{prefix}{wname}{c4}", bufs=1)
            nc.sync.dma_start(t[:], wd[c4 * 128:(c4 + 1) * 128, :])
            tiles.append(t)
        out[wname] = tiles
    for bname, bd in (("bq", bq_d), ("bk", bk_d)):
        tiles = []
        for oc in range(4):
            t = sb.tile([128, 1], F32, name=f"{prefix}{bname}{oc}", bufs=1)
            nc.sync.dma_start(t[:], bd[oc * 128:(oc + 1) * 128, :])
            tiles.append(t)
        out[bname] = tiles
    bvr = sb.tile([1, 512], BF16, name=f"{prefix}bvr", bufs=1)
    nc.sync.dma_start(bvr[:], bv_d[:])
    out["bvr"] = bvr
    return out


def _stage(nc, sb, ps, w, ones64, x_load, stag_for, residual_dst, res_pattern,
           after_chunk, prefix):
    """One attention stage over this core's 32 rows (16 chunks of 2 rows).

    x_load(cc, chunk, t): DMA 2-row pixel chunk of x into [128, 512] bf16 tile.
    stag_for(chunk, r, m) -> dst AP [128, 256] for the normalized output of
      head-pair m (channels m*128:(m+1)*128), row r of the chunk.
    residual_dst(chunk, cc) -> dst AP matching res_pattern's view of x.
    res_pattern: einops pattern for the 2-row x chunk residual view.
    after_chunk(chunk): hook (stores / A2A kick).
    """
    for chunk in range(16):
        x_t = []
        for cc in range(4):
            t = sb.tile([128, 512], BF16, name=f"{prefix}x{cc}",
                        tag=f"x{cc}", bufs=4)
            x_load(cc, chunk, t)
            x_t.append(t)
        xs = [x_t[cc][:] for cc in range(4)]
        # --- q/k projections: out [o-chunk 128, 512 pix], bias on copy ---
        q_sb, k_sb = [], []
        for wname, bname, dst in (("wq", "bq", q_sb), ("wk", "bk", k_sb)):
            for oc in range(4):
                pp = ps.tile([128, 512], F32, name="pp", tag="pp", bufs=2)
                for c4 in range(4):
                    nc.tensor.matmul(
                        pp[:],
                        w[wname][c4][:, oc * 128:(oc + 1) * 128],
                        xs[c4],
                        start=(c4 == 0), stop=(c4 == 3),
                    )
                t = sb.tile([128, 512], BF16, name=f"{wname}o{oc}",
                            tag=f"{wname}o", bufs=5)
                with nc.allow_low_precision(reason="bf16 q/k"):
                    if wname == "wq":
                        nc.scalar.activation(t[:], pp[:], IDENT,
                                             bias=w[bname][oc][:])
                    else:
                        nc.vector.tensor_scalar_add(t[:], pp[:],
                                                    w[bname][oc][:])
                dst.append(t)
        # --- v projected transposed [pix-chunk 128, 8 heads x 64] ---
        vT_sb = []
        for pc in range(4):
            pv = ps.tile([128, 512], F32, name="pp", tag="pp", bufs=2)
            for c4 in range(4):
                nc.tensor.matmul(
                    pv[:],
                    xs[c4][:, pc * 128:(pc + 1) * 128],
                    w["wv"][c4][:],
                    start=(c4 == 0), stop=(c4 == 3),
                )
            t = sb.tile([128, 512], BF16, name=f"vT{pc}", tag="vT", bufs=5)
            with nc.allow_low_precision(reason="bf16 v"):
                nc.vector.tensor_add(t[:], pv[:], w["bvt"][:])
            vT_sb.append(t)
        # --- attention per (row-in-chunk, head-pair) ---
        for r in range(2):
            for m in range(4):
                # QK^T both heads: at2[j', (h2, jh, i)]; K=64 row-tiled
                at2 = ps.tile([128, 1024], F32, name="at2", tag="at2", bufs=2)
                for h2 in range(2):
                    ph = h2 * 64
                    for jh in range(2):
                        nc.tensor.matmul(
                            at2[:, (h2 * 2 + jh) * 256:(h2 * 2 + jh + 1) * 256],
                            k_sb[m][ph:ph + 64,
                                    r * 256 + jh * 128:r * 256 + (jh + 1) * 128],
                            q_sb[m][ph:ph + 64, r * 256:(r + 1) * 256],
                            start=True, stop=True,
                        )
                e2 = sb.tile([128, 1024], BF16, name="e2", tag="e2", bufs=3)
                with nc.allow_low_precision(reason="bf16 attention weights"):
                    nc.scalar.activation(e2[:], at2[:], EXP)
                # AV + denominators for both heads in one [128, 512] bank:
                # rows h2*64.. = head h2; cols 0:256 AV, 256:512 denom
                ob = ps.tile([128, 512], F32, name="ob", tag="ob", bufs=2)
                for h2 in range(2):
                    po = h2 * 64
                    for jh in range(2):
                        nc.tensor.matmul(
                            ob[po:po + 64, 0:256],
                            vT_sb[2 * r + jh][:, (2 * m + h2) * 64:
                                              (2 * m + h2) * 64 + 64],
                            e2[:, (h2 * 2 + jh) * 256:(h2 * 2 + jh + 1) * 256],
                            start=(jh == 0), stop=(jh == 1),
                        )
                    for jh in range(2):
                        nc.tensor.matmul(
                            ob[po:po + 64, 256:512],
                            ones64[:, :],
                            e2[:, (h2 * 2 + jh) * 256:(h2 * 2 + jh + 1) * 256],
                            start=(jh == 0), stop=(jh == 1),
                        )
                r_sb = sb.tile([128, 256], F32, name="r_sb", tag="r_sb",
                               bufs=3)
                nc.vector.reciprocal(r_sb[:], ob[:, 256:512])
                with nc.allow_low_precision(reason="bf16 staging"):
                    nc.vector.tensor_mul(
                        stag_for(chunk, r, m), ob[:, 0:256], r_sb[:])
        # bulk residual on Pool
        for cc in range(4):
            dst = residual_dst(chunk, cc)
            src = xs[cc].rearrange(res_pattern, r=2)
            with nc.allow_low_precision(reason="bf16 staging"):
                nc.gpsimd.tensor_add(dst, dst, src)
        after_chunk(chunk)


def _build(variant="full"):
    ndev = 1 if variant == "sim1" else N_CORES
    nc = bacc.Bacc("TRN2", target_bir_lowering=False, debug=False,
                   num_devices=ndev)
    xi = nc.dram_tensor("xi", [D, PIX], BF16, kind="ExternalInput").ap()
    y = nc.dram_tensor("y", [D, RLOC, 256], BF16, kind="ExternalOutput").ap()
    w_ins = {}
    for p in ("1", "2"):
        ins = []
        for nm, shp, dt in (("wq", [D, D], BF16), ("wk", [D, D], BF16),
                            ("wv", [D, D], BF16), ("bq", [D, 1], F32),
                            ("bk", [D, 1], F32), ("bv", [1, D], BF16)):
            ins.append(nc.dram_tensor(nm + p, shp, dt, kind="ExternalInput").ap())
        w_ins[p] = ins

    with tile.TileContext(nc) as tc:
        with tc.tile_pool(name="sb", bufs=1) as sb, \
             tc.tile_pool(name="psum", bufs=1, space="PSUM") as psp, \
             tc.tile_pool(name="dram", bufs=1, space="DRAM") as dram:
            # A2A blocks: [dest j, c, l32 (j's share), s32 (sender's share)]
            a2a_in = dram.tile([N_CORES, D, RLOC, RLOC], BF16, name="a2a_in")
            a2a_out = dram.tile([N_CORES, D, RLOC, RLOC], BF16, name="a2a_out")

            ones64 = sb.tile([128, 64], BF16, name="ones64", bufs=1)
            nc.gpsimd.memset(ones64[:], 1.0)
            ones_row = sb.tile([1, 128], BF16, name="ones_row", bufs=1)
            nc.gpsimd.memset(ones_row[:], 1.0)
            w1 = _load_weights(nc, sb, "s1", w_ins["1"])
            w2 = _load_weights(nc, sb, "s2", w_ins["2"])
            # broadcast each stage's v-bias down 128 partitions (K=1 matmul)
            for sidx, wd in ((1, w1), (2, w2)):
                bvp = psp.tile([128, 512], F32, name="pp", tag="pp", bufs=2)
                nc.tensor.matmul(bvp[:], ones_row[0:1, :], wd["bvr"][:],
                                 start=True, stop=True)
                bvt = sb.tile([128, 512], BF16, name=f"bvt{sidx}",
                              tag=f"bvt{sidx}", bufs=1)
                with nc.allow_low_precision(reason="bf16 bias"):
                    nc.vector.tensor_copy(bvt[:], bvp[:])
                wd["bvt"] = bvt

            # ---- stage 1: row attention, S-sharded ----
            def x_load1(cc, chunk, t):
                nc.sync.dma_start(
                    t[:], xi[cc * 128:(cc + 1) * 128,
                             chunk * 512:(chunk + 1) * 512])

            # staging: per cc a [128, 256 l, 32 s] bf16 tile for the full
            # 32-row block; stored to a2a_in at stage end.
            stag1 = []
            for cc in range(4):
                t = sb.tile([128, 256, RLOC], BF16, name=f"stag1_{cc}",
                            tag=f"stg{cc}", bufs=1)
                stag1.append(t)

            def stag_for1(chunk, r, m):
                return stag1[m][:, :, chunk * 2 + r]

            def residual_dst1(chunk, cc):
                return stag1[cc][:, :, chunk * 2:chunk * 2 + 2]

            def after_chunk1(chunk):
                if chunk != 15:
                    return
                for cc in range(4):
                    for j in range(N_CORES):
                        nc.scalar.dma_start(
                            a2a_in[j, cc * 128:(cc + 1) * 128, :, :],
                            stag1[cc][:, j * 32:(j + 1) * 32, :])
                if variant == "sim1":
                    nc.gpsimd.dma_start(a2a_out[:], a2a_in[:])
                else:
                    nc.gpsimd.collective_compute(
                        "AllToAll", mybir.AluOpType.bypass,
                        replica_groups=[list(range(N_CORES))],
                        ins=[a2a_in.opt()], outs=[a2a_out.opt()],
                    )

            _stage(nc, sb, psp, w1, ones64, x_load1, stag_for1, residual_dst1,
                   "c (r l) -> c l r", after_chunk1, "s1")

            # ---- stage 2: column attention, L-sharded ----
            def x_load2(cc, chunk, t):
                for lr in range(2):
                    src = a2a_out[:, cc * 128:(cc + 1) * 128,
                                  chunk * 2 + lr, :] \
                        .rearrange("i c s -> c i s")
                    nc.sync.dma_start(
                        t[:, lr * 256:(lr + 1) * 256]
                        .rearrange("c (i s) -> c i s", i=8), src)

            # staging: per (cc, g) a [128, 8 l, 256 s] tile, stored to y
            # after every 4 chunks.
            stag2 = [[None] * 4 for _ in range(4)]

            def stag_for2(chunk, r, m):
                g = chunk // 4
                if stag2[m][g] is None:
                    for cc in range(4):
                        stag2[cc][g] = sb.tile(
                            [128, 8, 256], BF16, name=f"stag2_{cc}",
                            tag=f"stag2_{cc}", bufs=2)
                return stag2[m][g][:, (chunk % 4) * 2 + r, :]

            def residual_dst2(chunk, cc):
                g = chunk // 4
                return stag2[cc][g][:, (chunk % 4) * 2:(chunk % 4) * 2 + 2, :]

            def after_chunk2(chunk):
                if chunk % 4 != 3:
                    return
                g = chunk // 4
                for cc in range(4):
                    nc.scalar.dma_start(
                        y[cc * 128:(cc + 1) * 128, g * 8:(g + 1) * 8, :],
                        stag2[cc][g][:])

            _stage(nc, sb, psp, w2, ones64, x_load2, stag_for2, residual_dst2,
                   "c (r s) -> c r s", after_chunk2, "s2")

    nc.compile()
    return nc


def _get_nc(variant="full"):
    key = "nc:" + variant
    if key not in _CACHE:
        _CACHE[key] = _build(variant)
    return _CACHE[key]


def _in_maps(x, Wr, br, Wc, bc):
    import ml_dtypes
    bf = ml_dtypes.bfloat16
    x = np.asarray(x, dtype=np.float32)
    stage_w = {}
    for p, W, b in (("1", np.asarray(Wr, np.float32), np.asarray(br, np.float32)),
                    ("2", np.asarray(Wc, np.float32), np.asarray(bc, np.float32))):
        stage_w["wq" + p] = np.ascontiguousarray(W[0:D].T).astype(bf)
        stage_w["wk" + p] = np.ascontiguousarray(W[D:2 * D].T).astype(bf)
        stage_w["wv" + p] = np.ascontiguousarray(W[2 * D:3 * D].T).astype(bf)
        stage_w["bq" + p] = np.ascontiguousarray(b[0:D].reshape(D, 1))
        stage_w["bk" + p] = np.ascontiguousarray(b[D:2 * D].reshape(D, 1))
        stage_w["bv" + p] = np.ascontiguousarray(
            b[2 * D:3 * D].reshape(1, D)).astype(bf)
    maps = []
    for i in range(N_CORES):
        m = {"xi": np.ascontiguousarray(
            x[0, :, i * RLOC:(i + 1) * RLOC, :].reshape(D, PIX)).astype(bf)}
        m.update(stage_w)
        maps.append(m)
    return maps


def _get_runner(variant="full"):
    """Build (once) a cached jitted shard_map callable over the 8 cores."""
    rkey = "runner:" + variant
    if rkey in _CACHE:
        return _CACHE[rkey]
    import jax
    from jax.sharding import Mesh, PartitionSpec
    from jax.experimental.shard_map import shard_map
    from concourse import bass2jax as b2j

    nc = _get_nc(variant)
    b2j.install_neuronx_cc_hook()
    part_name = nc.partition_id_tensor.name if nc.partition_id_tensor else None
    in_names, out_names, out_avals, zero_outs = [], [], [], []
    for alloc in nc.m.functions[0].allocations:
        if not isinstance(alloc, mybir.MemoryLocationSet):
            continue
        name = alloc.memorylocations[0].name
        if alloc.kind == "ExternalInput":
            if name != part_name:
                in_names.append(name)
        elif alloc.kind == "ExternalOutput":
            out_names.append(name)
            shape = tuple(alloc.tensor_shape)
            dtype = mybir.dt.np(alloc.dtype)
            out_avals.append(jax.core.ShapedArray(shape, dtype))
            zero_outs.append(np.zeros(shape, dtype))
    n_params = len(in_names)
    all_names = in_names + out_names
    if part_name is not None:
        all_names = all_names + [part_name]

    def _body(*args):
        operands = list(args)
        if part_name is not None:
            operands.append(b2j.partition_id_tensor())
        outs = b2j._bass_exec_p.bind(
            *operands,
            out_avals=tuple(out_avals),
            in_names=tuple(all_names),
            out_names=tuple(out_names),
            lowering_input_output_aliases=(),
            sim_require_finite=True,
            sim_require_nnan=True,
            nc=nc,
        )
        return tuple(outs)

    devices = jax.devices()[:N_CORES]
    mesh = Mesh(np.asarray(devices), ("core",))
    specs = (PartitionSpec("core"),) * (n_params + len(out_names))
    sharded = jax.jit(
        shard_map(_body, mesh=mesh, in_specs=specs,
                  out_specs=(PartitionSpec("core"),) * len(out_names),
                  check_rep=False),
        keep_unused=True,
    )
    concat_zeros = [
        jax.device_put(
            np.zeros((N_CORES * z.shape[0], *z.shape[1:]), z.dtype),
            jax.sharding.NamedSharding(mesh, PartitionSpec("core")))
        for z in zero_outs
    ]
    _CACHE[rkey] = (sharded, in_names, out_names, out_avals, concat_zeros)
    return _CACHE[rkey]


def _run(maps):
    sharded, in_names, out_names, out_avals, concat_zeros = _get_runner()
    concat_in = [
        np.concatenate([maps[c][nm] for c in range(N_CORES)], axis=0)
        for nm in in_names
    ]
    out_arrs = sharded(*concat_in, *concat_zeros)
    return [
        {nm: np.asarray(out_arrs[i]).reshape(N_CORES, *out_avals[i].shape)[c]
         for i, nm in enumerate(out_names)}
        for c in range(N_CORES)
    ]


def kernel(x, Wr, br, Wc, bc):
    maps = _in_maps(x, Wr, br, Wc, bc)
    results = _run(maps)
    # per-core y is [c, l32, s256] bf16; transpose to [c, s, l] and concat l
    cols = [np.transpose(results[i]["y"].astype(np.float32), (0, 2, 1))
            for i in range(N_CORES)]
    out = np.concatenate(cols, axis=2)
    return out[None]
